# revision 17
# baseline (speedup 1.0000x reference)
"""Multi-head causal attention (B=2, T=2048, D=1024, H=16) on 8 Trainium2
NeuronCores.

Sharding: core c handles batch b = c//4 and head group g = c%4 (4 heads,
o-columns [256g, 256g+256)).  Host pre-transposes x and the weight slices so
every matmul operand arrives in contraction-major layout; each core computes
its partial output projection y_part = att_part @ W_o.T[cols] and the host
sums the 4 partials per batch and adds b_o.

v2: all matmul operands in bf16 (FWL weight loads, no f32r small-N penalty),
softmax denominator replicated across 64 PSUM partitions via a widened ones
block in the PV stationary ([V_h | 1*64]), fast approximate reciprocal, and
projection / output-projection chunks interleaved into the attention loop so
the PE stays busy while the scalar engine runs exp.

v3 (schedule rework, from NTFF trace of v2):
  - reciprocal_approx_fast instead of InstReciprocal (3.3us -> 0.7us x16),
    unblocking the PSUM "PO" ring at head-pair boundaries.
  - ACT does exp ONLY; Q/K bias add, V copy and y copy move to the DVE
    (tensor_scalar_add / tensor_copy).
  - k-tiles reordered: full prior-span blocks first, masked diagonal
    blocks last, so the gpsimd mask multiply is off the exp->PV path.
  - filler chunks paced per span (proj chunks early spans, ALL output-
    projection chunks deferred to the ACT-bound last span) so the PE
    never idles >3.4us and the HAM clock gate stays at 8/8.
  - dummy exp at t=0 (hides ACT_TABLE_LOAD) and dummy matmuls during the
    initial x DMA (pre-warms the PE clock).

Per-core device program:
  Q^T,K^T  [o,t] = wT.T @ x^T    (Q prescaled by 1/sqrt(64), biases folded)
  V        [t,o] = x^T.T @ wvT   (per head: [V | ones*64] for softmax denom)
  S^T      [k,q] = K^T_h.T @ Q^T_h   (two heads concurrent in PE row groups)
  P = exp(S^T)   (causal: lower-tri blocks only, diag blocks masked)
  O^T[0:64,q], D[64:128,q] = [V_h|1].T @ P
  att^T = O^T * (1/D)                (rowwise DVE multiply)
  y_part[t,:] = att^T.T @ woT
"""
import sys

for _p in ("/opt/trn_rl_repo", "/root/.axon_site/_ro/trn_rl_repo"):
    if _p not in sys.path:
        sys.path.insert(0, _p)

from collections import deque

import numpy as np
import ml_dtypes

import concourse.bass as bass
import concourse.tile as tile
from concourse import bacc, mybir

F32 = mybir.dt.float32
BF16 = mybir.dt.bfloat16
F8 = mybir.dt.float8e4
NPBF16 = ml_dtypes.bfloat16

N_CORES = 8
EMBED = 1024
NH_CORE = 4          # heads per core
DH = 64              # head dim
OC = NH_CORE * DH    # 256 o-columns per core
KC = EMBED // 128    # 8 contraction chunks
NO = OC // 128       # 2 o-tiles of 128


def build_body(tc, aps, T, skip=()):
    nc = tc.nc
    P = 128
    SPAN = 512
    NSPAN = T // SPAN
    TPS = SPAN // P      # k/q tiles per span
    NT = T // P

    xT, wqT, wkT, wvT, woT, bq, bk, bv, y = (
        aps["xT"], aps["wqT"], aps["wkT"], aps["wvT"], aps["woT"],
        aps["bq"], aps["bk"], aps["bv"], aps["y"],
    )

    sb = aps["sb_pool"]
    ps = aps["ps_pool"]

    Exp = mybir.ActivationFunctionType.Exp
    Ident = mybir.ActivationFunctionType.Identity

    # ---- input loads (Q/K weights first: first PE chunks need them) ----
    wq_sb = sb.tile([128, KC, OC], BF16, tag="wq")
    nc.sync.dma_start(wq_sb[:], wqT.rearrange("(kc p) o -> p kc o", p=P))
    wk_sb = sb.tile([128, KC, OC], BF16, tag="wk")
    nc.sync.dma_start(wk_sb[:], wkT.rearrange("(kc p) o -> p kc o", p=P))
    bq_sb = sb.tile([128, NO], F32, tag="bq")
    nc.sync.dma_start(bq_sb[:], bq.rearrange("(mo p) -> p mo", p=P))
    bk_sb = sb.tile([128, NO], F32, tag="bk")
    nc.sync.dma_start(bk_sb[:], bk.rearrange("(mo p) -> p mo", p=P))
    wv_sb = sb.tile([128, KC, OC], BF16, tag="wv")
    nc.sync.dma_start(wv_sb[:], wvT.rearrange("(kc p) o -> p kc o", p=P))
    bv_sb = sb.tile([1, OC], BF16, tag="bv")
    nc.sync.dma_start(bv_sb[:], bv.rearrange("(a o) -> a o", a=1))
    xc = []
    for kc in range(KC):
        t = sb.tile([128, T], BF16, tag="xt", bufs=KC)
        nc.sync.dma_start(t[:], xT.rearrange("(kc p) t -> kc p t", p=P)[kc])
        xc.append(t)
    wo_sb = sb.tile([128, NO, EMBED], BF16, tag="wo")
    nc.sync.dma_start(wo_sb[:], woT.rearrange("(kc p) o -> p kc o", p=P))

    # ---- constants ----
    ones_f = sb.tile([128, 256], F32, tag="onesf")
    nc.gpsimd.memset(ones_f[:], 1.0)
    ones_bf = sb.tile([128, 256], BF16, tag="ones")
    nc.vector.tensor_copy(ones_bf[:], ones_f[:])
    # warm the ACT exp table (~2.7us) while the input DMAs stream in
    warm_exp = sb.tile([128, 8], F32, tag="wexp")
    nc.scalar.activation(warm_exp[:], ones_f[:, 0:8], Exp)
    # keep the PE busy during the x DMA so the HAM clock gate reaches 8/8
    # before the first real matmul chunk (idle >3.4us would re-throttle)
    for _ in range(20):
        wmm = ps.tile([128, 512], F32, tag="B", bufs=2)
        nc.tensor.matmul(wmm[:, 0:256], ones_bf[:, 0:128], ones_bf[:],
                         start=True, stop=True)

    QT_sb = sb.tile([128, NO, T], BF16, tag="qt")
    KT_sb = sb.tile([128, NO, T], BF16, tag="kt")
    # per head slot of 128: [ones (0:64) | V (64:128)] -- D lands on psum
    # partitions 0:63 where the full-width custom recip is safe
    V_sb = sb.tile([128, NT, NH_CORE * 128], BF16, tag="v")
    attT_sb = sb.tile([128, NO, T], BF16, tag="att")
    # the ones blocks are constant: fill once on the Pool engine
    nc.gpsimd.memset(
        V_sb[:].rearrange("p nt (h c) -> p nt h c", c=128)[:, :, :, 0:DH], 1.0,
    )

    # ---------------- emission helpers ----------------

    def qk_chunk(s, dst, wsb, bias_sb, mo):
        def emit():
            pt = ps.tile([128, 512], F32, tag="B", bufs=2)
            for kc in range(KC):
                nc.tensor.matmul(
                    pt[:],
                    wsb[:, kc, mo * 128:(mo + 1) * 128],
                    xc[kc][:, s * SPAN:(s + 1) * SPAN],
                    start=(kc == 0), stop=(kc == KC - 1),
                )
            with nc.allow_low_precision(reason="bf16 qkt from f32 psum"):
                nc.vector.tensor_scalar_add(
                    dst[:, mo, s * SPAN:(s + 1) * SPAN], pt[:],
                    bias_sb[:, mo:mo + 1],
                )
        return emit

    def v_chunk(ti):
        def emit():
            pv = ps.tile([128, 512], F32, tag="B", bufs=2)
            for kc in range(KC):
                nc.tensor.matmul(
                    pv[:, 0:OC], xc[kc][:, ti * 128:(ti + 1) * 128],
                    wv_sb[:, kc, :],
                    start=(kc == 0), stop=False,
                )
            nc.tensor.matmul(
                pv[:, 0:OC], ones_bf[0:1, 0:128], bv_sb[0:1, :],
                start=False, stop=True,
            )
            vview = V_sb[:, ti, :].rearrange("p (h c) -> p h c", c=128)
            with nc.allow_low_precision(reason="bf16 v from f32 psum"):
                nc.vector.tensor_copy(
                    vview[:, :, DH:128],
                    pv[:, 0:OC].rearrange("p (h d) -> p h d", d=DH),
                )
        return emit

    def y_chunk(ti, no):
        def emit():
            py = ps.tile([128, 512], F32, tag="B", bufs=2)
            for kc2 in range(NO):
                nc.tensor.matmul(
                    py[:],
                    attT_sb[:, kc2, ti * 128:(ti + 1) * 128],
                    wo_sb[:, kc2, no * 512:(no + 1) * 512],
                    start=(kc2 == 0), stop=(kc2 == NO - 1),
                )
            ysb = sb.tile([128, 512], F32, tag="y", bufs=3)
            nc.vector.tensor_copy(ysb[:], py[:])
            nc.sync.dma_start(
                y[ti * 128:(ti + 1) * 128, no * 512:(no + 1) * 512], ysb[:],
            )
        return emit

    def proj_chunks(s):
        out = []
        for dst, wsb, bias_sb in ((QT_sb, wq_sb, bq_sb), (KT_sb, wk_sb, bk_sb)):
            for mo in range(NO):
                out.append(qk_chunk(s, dst, wsb, bias_sb, mo))
        for ti in range(TPS * s, TPS * (s + 1)):
            out.append(v_chunk(ti))
        return out

    # ---------------- filler pacing ----------------
    # One global queue; span s drains at `rate` fillers per attention
    # iteration. Supply: span s queues proj(s+1); the ACT-bound last span
    # (and the tail) takes every output-projection chunk.
    fill = deque()
    credit = 0.0

    def drain(rate):
        nonlocal credit
        credit += rate
        while credit >= 1.0 and fill:
            fill.popleft()()
            credit -= 1.0

    # ---------------- main schedule ----------------

    # span-0 Q/K projections kc-major: the first matmuls need only wq/wk +
    # xc[0], so the PE starts real work as soon as the first x chunk lands
    # instead of waiting for the full x DMA.
    combos = [(QT_sb, wq_sb, bq_sb, 0), (QT_sb, wq_sb, bq_sb, 1),
              (KT_sb, wk_sb, bk_sb, 0), (KT_sb, wk_sb, bk_sb, 1)]
    ptiles = [ps.tile([128, 512], F32, tag=("B", "B", "A", "A")[j], bufs=2,
                      name=f"pqk0_{j}") for j in range(4)]
    for kc in range(KC):
        for j, (dst, wsb, bias_sb, mo) in enumerate(combos):
            nc.tensor.matmul(
                ptiles[j][:], wsb[:, kc, mo * 128:(mo + 1) * 128],
                xc[kc][:, 0:SPAN], start=(kc == 0), stop=(kc == KC - 1),
            )
    for j, (dst, wsb, bias_sb, mo) in enumerate(combos):
        with nc.allow_low_precision(reason="bf16 qkt from f32 psum"):
            nc.vector.tensor_scalar_add(
                dst[:, mo, 0:SPAN], ptiles[j][:], bias_sb[:, mo:mo + 1])
    for ti in range(TPS):
        v_chunk(ti)()

    for s in range(NSPAN):
        n_iter = 2 * (TPS * (s + 1))          # attention iterations this span
        if s + 1 < NSPAN:
            fill.extend(proj_chunks(s + 1))
        if s == NSPAN - 1:
            for ti in range(0, TPS * s):      # all y chunks for spans 0..s-1
                for no in range(EMBED // 512):
                    fill.append(y_chunk(ti, no))
        rate = min(1.0, len(fill) / max(n_iter, 1))
        credit = 0.0

        # ---- attention for this span: head pairs share the PE array ----
        for hp in range(NH_CORE // 2 if "att" not in skip else 0):
            poh = [ps.tile([128, 512], F32, tag="PO", bufs=2,
                           name=f"po_{s}_{hp}_{i}") for i in range(2)]
            # full prior-span blocks first; masked diagonal blocks last so
            # the mask never gates the next PV directly
            n_prior = TPS * s
            kts = list(range(0, n_prior)) + list(range(n_prior, n_prior + TPS))

            pend_pv = deque()
            state = {"first": True}

            def emit_one_pv(last):
                kt, lo, pb = pend_pv.popleft()
                for hh in range(2 if "pv" not in skip else 0):
                    h = 2 * hp + hh
                    nc.tensor.matmul(
                        poh[hh][:, lo:SPAN],
                        V_sb[:, kt, 128 * h:128 * (h + 1)],
                        pb[:, hh, lo:SPAN],
                        start=state["first"], stop=last,
                    )
                state["first"] = False

            for idx, kt in enumerate(kts):
                prior = kt < n_prior
                lo = 0 if prior else 128 * (kt - n_prior)
                pstile = ps.tile([128, 2, 512], F32, tag="A", bufs=2)
                for hh in range(2 if "smm" not in skip else 0):
                    h = 2 * hp + hh
                    bp = 64 * (h % 2)
                    nc.tensor.matmul(
                        pstile[:, hh, lo:SPAN],
                        KT_sb[bp:bp + DH, hp, kt * 128:(kt + 1) * 128],
                        QT_sb[bp:bp + DH, hp, SPAN * s + lo:SPAN * (s + 1)],
                        start=True, stop=True,
                    )
                pb = sb.tile([128, 2, SPAN], BF16, tag="p", bufs=6)
                if "exp" not in skip:
                    nc.scalar.activation(
                        pb[:, :, lo:SPAN], pstile[:, :, lo:SPAN], Exp,
                    )
                if not prior and "mask" not in skip:
                    # causal mask on the diagonal block: keep q >= k
                    blk = pb[:, :, lo:lo + 128]
                    nc.gpsimd.affine_select(
                        out=blk, in_=blk,
                        compare_op=mybir.AluOpType.is_ge, fill=0.0,
                        base=0, pattern=[[0, 2], [1, 128]],
                        channel_multiplier=-1,
                    )
                drain(rate)
                # PV trails by two so the PE never reaches an exp-gated PV
                # before the ACT engine has finished producing it
                pend_pv.append((kt, lo, pb))
                if len(pend_pv) > 2:
                    emit_one_pv(last=False)
            while pend_pv:
                emit_one_pv(last=(len(pend_pv) == 1))
            # normalize: att^T = O^T * (1 / D); approx recip (~51 ULP) keeps
            # the PO psum ring free for the next head pair
            for hh in range(2):
                h = 2 * hp + hh
                bp = 64 * (h % 2)
                rb = sb.tile([128, SPAN], F32, tag="rb", bufs=2)
                with nc.allow_low_precision(reason="f32 recip of f32 psum"):
                    # full-128-partition custom op: partitions 0:63 hold 1/D;
                    # 64:128 compute 1/O^T, never read (custom DVE ops mis-
                    # execute on sub-128 partition ranges on this HW)
                    nc.vector.reciprocal_approx_fast(rb[:], poh[hh][:])
                    nc.vector.tensor_mul(
                        attT_sb[bp:bp + DH, hp, SPAN * s:SPAN * (s + 1)],
                        poh[hh][64:128, :],
                        rb[0:64, :],
                    )

        # everything queued for the next span must land before it starts
        while fill:
            fill.popleft()()

    # ---- remaining output projection ----
    for ti in range(TPS * (NSPAN - 1), TPS * NSPAN):
        for no in range(EMBED // 512):
            y_chunk(ti, no)()


def build_nc(T=2048, reps=1, skip=()):
    nc = bacc.Bacc("TRN2", target_bir_lowering=False, debug=False,
                   enable_asserts=False, num_devices=N_CORES)
    aps = {
        "xT": nc.dram_tensor("xT", (EMBED, T), BF16, kind="ExternalInput").ap(),
        "wqT": nc.dram_tensor("wqT", (EMBED, OC), BF16, kind="ExternalInput").ap(),
        "wkT": nc.dram_tensor("wkT", (EMBED, OC), BF16, kind="ExternalInput").ap(),
        "wvT": nc.dram_tensor("wvT", (EMBED, OC), BF16, kind="ExternalInput").ap(),
        "woT": nc.dram_tensor("woT", (OC, EMBED), BF16, kind="ExternalInput").ap(),
        "bq": nc.dram_tensor("bq", (OC,), F32, kind="ExternalInput").ap(),
        "bk": nc.dram_tensor("bk", (OC,), F32, kind="ExternalInput").ap(),
        "bv": nc.dram_tensor("bv", (OC,), BF16, kind="ExternalInput").ap(),
        "y": nc.dram_tensor("y", (T, EMBED), F32, kind="ExternalOutput").ap(),
    }
    with tile.TileContext(nc) as tc:
        with tc.tile_pool(name="sb", bufs=1) as sb, \
             tc.tile_pool(name="ps", bufs=1, space="PSUM") as ps:
            aps["sb_pool"] = sb
            aps["ps_pool"] = ps
            if reps == 1:
                build_body(tc, aps, T, skip=skip)
            else:
                hints = (mybir.EngineType.PE, mybir.EngineType.Activation,
                         mybir.EngineType.DVE, mybir.EngineType.SP,
                         mybir.EngineType.Pool)
                with tc.For_i(0, reps, 1, hint_engines=hints):
                    build_body(tc, aps, T, skip=skip)
    nc.compile()
    return nc


def shard_inputs(x, W_q, b_q, W_k, b_k, W_v, b_v, W_o, b_o=None):
    """Full inputs -> list of 8 per-core input dicts."""
    in_maps = []
    for c in range(N_CORES):
        b, g = divmod(c, 4)
        sl = slice(OC * g, OC * (g + 1))
        in_maps.append({
            "xT": np.ascontiguousarray(x[b].T).astype(NPBF16),
            "wqT": np.ascontiguousarray((W_q[sl, :] * 0.125).T).astype(NPBF16),
            "wkT": np.ascontiguousarray(W_k[sl, :].T).astype(NPBF16),
            "wvT": np.ascontiguousarray(W_v[sl, :].T).astype(NPBF16),
            "woT": np.ascontiguousarray(W_o[:, sl].T).astype(NPBF16),
            "bq": np.ascontiguousarray(b_q[sl] * 0.125).astype(np.float32),
            "bk": np.ascontiguousarray(b_k[sl]).astype(np.float32),
            "bv": np.ascontiguousarray(b_v[sl]).astype(NPBF16),
        })
    return in_maps


def _make_runner(nc, n_cores=N_CORES):
    """Compile-once, run-many SPMD runner (mirrors bass2jax.run_bass_via_pjrt)."""
    import jax
    from jax.sharding import Mesh, PartitionSpec
    from jax.experimental.shard_map import shard_map
    from concourse import bass2jax

    bass2jax.install_neuronx_cc_hook()
    partition_name = nc.partition_id_tensor.name if nc.partition_id_tensor else None
    in_names, out_names, out_avals, zero_outs = [], [], [], []
    for alloc in nc.m.functions[0].allocations:
        if not isinstance(alloc, mybir.MemoryLocationSet):
            continue
        name = alloc.memorylocations[0].name
        if alloc.kind == "ExternalInput":
            if name != partition_name:
                in_names.append(name)
        elif alloc.kind == "ExternalOutput":
            out_names.append(name)
            shape = tuple(alloc.tensor_shape)
            dtype = mybir.dt.np(alloc.dtype)
            out_avals.append(jax.core.ShapedArray(shape, dtype))
            zero_outs.append(np.zeros(shape, dtype))
    n_params = len(in_names)
    n_outs = len(out_avals)
    in_names_all = list(in_names) + list(out_names)
    if partition_name is not None:
        in_names_all.append(partition_name)
    donate = tuple(range(n_params, n_params + n_outs))

    def _body(*args):
        operands = list(args)
        if partition_name is not None:
            operands.append(bass2jax.partition_id_tensor())
        outs = bass2jax._bass_exec_p.bind(
            *operands,
            out_avals=tuple(out_avals),
            in_names=tuple(in_names_all),
            out_names=tuple(out_names),
            lowering_input_output_aliases=(),
            sim_require_finite=True,
            sim_require_nnan=True,
            nc=nc,
        )
        return tuple(outs)

    devices = jax.devices()[:n_cores]
    assert len(devices) == n_cores
    mesh = Mesh(np.asarray(devices), ("core",))
    in_specs = (PartitionSpec("core"),) * (n_params + n_outs)
    out_specs = (PartitionSpec("core"),) * len(out_names)
    jitted = jax.jit(
        shard_map(_body, mesh=mesh, in_specs=in_specs, out_specs=out_specs,
                  check_rep=False),
        donate_argnums=donate, keep_unused=True,
    )

    from jax.sharding import NamedSharding

    class Runner:
        def __init__(self):
            self._in_dev = None
            self._out_dev = None

        def prepare(self, in_maps):
            per_core = [[np.asarray(m[name]) for name in in_names]
                        for m in in_maps]
            concat_in = [
                np.concatenate([per_core[c][i] for c in range(n_cores)], axis=0)
                for i in range(n_params)
            ]
            sh = NamedSharding(mesh, PartitionSpec("core"))
            self._in_dev = [jax.device_put(a, sh) for a in concat_in]
            concat_zeros = [np.concatenate([z] * n_cores, axis=0)
                            for z in zero_outs]
            self._out_dev = [jax.device_put(a, sh) for a in concat_zeros]
            for a in self._in_dev + self._out_dev:
                a.block_until_ready()

        def execute(self):
            outs = jitted(*self._in_dev, *self._out_dev)
            for a in outs:
                a.block_until_ready()
            self._out_dev = list(outs)

        def fetch(self):
            out_arrs = [np.asarray(a) for a in self._out_dev]
            results = []
            for c in range(n_cores):
                m = {}
                for i, name in enumerate(out_names):
                    per_len = out_arrs[i].shape[0] // n_cores
                    m[name] = out_arrs[i][c * per_len:(c + 1) * per_len]
                results.append(m)
            return results

        def run(self, in_maps):
            self.prepare(in_maps)
            self.execute()
            return self.fetch()

    return Runner()


_CACHE = {}


def _get_runner(T=2048, reps=1):
    key = (T, reps)
    if key not in _CACHE:
        nc = build_nc(T=T, reps=reps)
        _CACHE[key] = _make_runner(nc)
    return _CACHE[key]


def kernel(**inputs):
    inputs = {k: np.asarray(v, dtype=np.float32) for k, v in inputs.items()}
    x = inputs["x"]
    B, T, C = x.shape
    in_maps = shard_inputs(**inputs)
    runner = _get_runner(T=T, reps=1)
    results = runner.run(in_maps)
    out = np.zeros((B, T, C), dtype=np.float32)
    for c in range(N_CORES):
        out[c // 4] += results[c]["y"]
    out += inputs["b_o"]
    return out


if __name__ == "__main__":
    rng = np.random.default_rng(0)
    s = 1.0 / np.sqrt(EMBED)
    ins = {
        "x": rng.standard_normal((2, 2048, EMBED), dtype=np.float32),
        "W_q": rng.uniform(-s, s, (EMBED, EMBED)).astype(np.float32),
        "b_q": rng.uniform(-s, s, (EMBED,)).astype(np.float32),
        "W_k": rng.uniform(-s, s, (EMBED, EMBED)).astype(np.float32),
        "b_k": rng.uniform(-s, s, (EMBED,)).astype(np.float32),
        "W_v": rng.uniform(-s, s, (EMBED, EMBED)).astype(np.float32),
        "b_v": rng.uniform(-s, s, (EMBED,)).astype(np.float32),
        "W_o": rng.uniform(-s, s, (EMBED, EMBED)).astype(np.float32),
        "b_o": rng.uniform(-s, s, (EMBED,)).astype(np.float32),
    }
    out = kernel(**ins)
    print("kernel out", out.shape, out.dtype, float(np.abs(out).max()))

